# revision 6
# baseline (speedup 1.0000x reference)
"""GATv2 2-layer GNN (N=50000, E=500000, 512->4x64->64, log_softmax)
on 8 TRN2 NeuronCores via Bass/Tile.

Sharding: nodes relabeled by (in-degree, lo-src-count) and degree-striped so
core c owns the contiguous relabeled range [c*6272, (c+1)*6272) with balanced
load; edges sharded by destination. L1 node transform xl (the per-edge gather
table) is computed replicated on every core in bf16 (cheaper than an
all-gather through the slow ncfw collectives); xr only for local nodes.
Per-edge work uses a degree-padded layout (dst node on partition, its edges
along free), fetching source rows with dma_gather (lo/hi split tables for the
signed-int16 index range). Segment softmax uses a global exp-shift. The small
L2 table (xl2 = h @ W2l) is exchanged with one ncfw AllGather.
"""

import numpy as np
import ml_dtypes

import concourse.bass as bass
import concourse.bacc as bacc
import concourse.tile as tile
from concourse import mybir
from concourse import library_config
from concourse.bass_utils import run_bass_kernel_spmd

P = 128
N, E = 50000, 500000
DIN, DH, H1, DOUT = 512, 64, 4, 64
HID = DH * H1  # 256
NEG = 0.2
NC = 8
NP = 50176
NPC = NP // NC  # 6272
NSTRIPE = NPC // P  # 49
NBLK = NP // P  # 392
SPLIT = 32768
SHIFT = 8.0
CHUNK_COLS = 16  # gather-slab columns per chunk
GW = 4  # edge-op column-group width
XCH_BLKS = 7  # node blocks per x-stream chunk
BF = mybir.dt.bfloat16
F32 = mybir.dt.float32
NPBF = ml_dtypes.bfloat16
CAP = 1


def _apx(a, dims):
    return bass.AP(a.tensor, a.offset, [list(a.ap[0])] + [list(d) for d in dims])


def _spill_waits(nc, cap=CAP):
    n = 0
    for f in nc.m.functions:
        for bb in f.blocks:
            out = []
            for inst in list(bb.instructions):
                si = inst.sync_info
                if si is not None and len(si.on_wait) > cap:
                    waits = list(si.on_wait)
                    excess, keep = waits[:-cap], waits[-cap:]
                    for ci in range(0, len(excess), cap):
                        out.append(
                            mybir.InstNoOp(
                                name=f"{inst.name}-wsp-{n}",
                                engine=inst.engine,
                                sync_info=mybir.SyncInfo(
                                    on_wait=excess[ci : ci + cap], on_update=[]
                                ),
                            )
                        )
                        n += 1
                    inst.sync_info = mybir.SyncInfo(
                        on_wait=keep, on_update=list(si.on_update)
                    )
                out.append(inst)
            bb.instructions = out
    return n


def _host_prep(edge_index):
    src0 = edge_index[0].astype(np.int64)
    dst0 = edge_index[1].astype(np.int64)
    deg = np.bincount(dst0, minlength=NP)

    # pass 1: degree sort to estimate lo-counts, pass 2: (deg, klo) sort
    order1 = np.argsort(deg, kind="stable")
    new1 = np.empty(NP, dtype=np.int64)
    new1[order1] = np.arange(NP)
    klo1 = np.bincount(dst0[new1[src0] < SPLIT], minlength=NP)  # by old id
    rank2 = np.lexsort((klo1[order1], deg[order1]))  # over order1 positions
    old_of_rank = order1[rank2]

    g = np.arange(NP) // P
    pp = np.arange(NP) % P
    core = g % NC
    stripe = g // NC
    nid_of_rank = core * NPC + stripe * P + pp
    old_of_nid = np.empty(NP, dtype=np.int64)
    old_of_nid[nid_of_rank] = old_of_rank
    nid_of_old = np.empty(NP, dtype=np.int64)
    nid_of_old[old_of_nid] = np.arange(NP)

    srcn = nid_of_old[src0]
    dstn = nid_of_old[dst0]
    lo = srcn < SPLIT

    dcore = dstn // NPC
    dstripe = (dstn % NPC) // P
    dpart = dstn % P

    klo_n = np.bincount(dstn[lo], minlength=NP).reshape(NC, NSTRIPE, P)
    khi_n = np.bincount(dstn[~lo], minlength=NP).reshape(NC, NSTRIPE, P)
    Dlo = klo_n.max(axis=(0, 2)).astype(np.int64)
    Dhi = khi_n.max(axis=(0, 2)).astype(np.int64)

    # chunk packing (sum of lo+hi cols <= CHUNK_COLS)
    chunks = []
    s = 0
    Dtot = Dlo + Dhi
    while s < NSTRIPE:
        e = s
        cols = 0
        while e < NSTRIPE and cols + Dtot[e] <= CHUNK_COLS:
            cols += int(Dtot[e])
            e += 1
        if e == s:
            e, cols = s + 1, int(Dtot[s])
        chunks.append((s, e, cols))
        s = e

    # per-chunk column layout: lo cols of stripes s0..s1-1, then hi cols
    stripe_lo_off = {}
    stripe_hi_off = {}
    chunk_lo_cols = []
    chunk_hi_cols = []
    for ci, (s0, s1, cols) in enumerate(chunks):
        o = 0
        for j in range(s0, s1):
            stripe_lo_off[j] = o
            o += int(Dlo[j])
        lo_cols = o
        for j in range(s0, s1):
            stripe_hi_off[j] = o
            o += int(Dhi[j])
        chunk_lo_cols.append(lo_cols)
        chunk_hi_cols.append(o - lo_cols)

    # per-core slot arrays
    eorder = np.lexsort((dpart, dstripe, dcore))
    srcs, dc, dj, dp, loe = (
        srcn[eorder],
        dcore[eorder],
        dstripe[eorder],
        dpart[eorder],
        lo[eorder],
    )
    per_core = []
    tot_cols = sum(c for (_, _, c) in chunks)
    for c in range(NC):
        m = dc == c
        cs, cj, cp, cl = srcs[m], dj[m], dp[m], loe[m]
        grid = np.zeros((P, tot_cols), dtype=np.int32)
        mask = np.zeros((P, tot_cols), dtype=np.float32)
        # chunk base col for each stripe
        chunk_base = {}
        b = 0
        for ci, (s0, s1, cols) in enumerate(chunks):
            for j in range(s0, s1):
                chunk_base[j] = b
            b += cols
        # rank within (stripe, part, lo-ness)
        key = cj * (P * 2) + cp * 2 + (~cl).astype(np.int64)
        srt = np.argsort(key, kind="stable")
        ks = key[srt]
        first = np.r_[True, ks[1:] != ks[:-1]]
        gs_ = np.flatnonzero(first)
        sizes = np.diff(np.r_[gs_, len(ks)])
        rnk = np.arange(len(ks)) - np.repeat(gs_, sizes)
        rank = np.empty(len(ks), dtype=np.int64)
        rank[srt] = rnk
        base = np.array([chunk_base[j] for j in range(NSTRIPE)])
        off_lo = np.array([stripe_lo_off[j] for j in range(NSTRIPE)])
        off_hi = np.array([stripe_hi_off[j] for j in range(NSTRIPE)])
        col = np.where(
            cl, base[cj] + off_lo[cj] + rank, base[cj] + off_hi[cj] + rank
        )
        grid[cp, col] = np.where(cl, cs, cs - SPLIT)
        mask[cp, col] = 1.0
        per_core.append((grid, mask))

    meta = dict(
        chunks=chunks,
        Dlo=Dlo,
        Dhi=Dhi,
        stripe_lo_off=stripe_lo_off,
        stripe_hi_off=stripe_hi_off,
        chunk_lo_cols=chunk_lo_cols,
        chunk_hi_cols=chunk_hi_cols,
        tot_cols=tot_cols,
        old_of_nid=old_of_nid,
        nid_of_old=nid_of_old,
    )
    return per_core, meta


def _wrap_idx_streams(grid, meta):
    """grid [P, tot_cols] -> (idx_lo_i16 [P, slots_lo/16], idx_hi_i16) in
    dma_gather wrapped layout; slot i of a chunk-stream at [i%16, i//16],
    replicated x8 partition groups."""
    chunks = meta["chunks"]
    lo_parts, hi_parts = [], []
    b = 0
    for ci, (s0, s1, cols) in enumerate(chunks):
        lc, hc = meta["chunk_lo_cols"][ci], meta["chunk_hi_cols"][ci]
        blk = grid[:, b : b + cols]
        lo_parts.append(blk[:, :lc].T.reshape(-1))  # slot i = col*128+p
        hi_parts.append(blk[:, lc : lc + hc].T.reshape(-1))
        b += cols
    def wrap(flat):
        n = flat.shape[0]
        w = flat.reshape(n // 16, 16).T
        return np.ascontiguousarray(np.tile(w, (NC, 1))).astype(np.int16)
    return wrap(np.concatenate(lo_parts)), wrap(np.concatenate(hi_parts))


def _gen_nc(meta):
    chunks = meta["chunks"]
    Dlo, Dhi = meta["Dlo"], meta["Dhi"]
    tot_cols = meta["tot_cols"]
    slots_lo = sum(c for c in meta["chunk_lo_cols"]) * P
    slots_hi = sum(c for c in meta["chunk_hi_cols"]) * P
    max_cols = max(c for (_, _, c) in chunks)

    nc = bacc.Bacc(
        "TRN2",
        target_bir_lowering=False,
        debug=False,
        num_devices=NC,
        num_swdge_queues=4,
    )

    xT = nc.dram_tensor("xT", [DIN, NP], BF, kind="ExternalInput")
    xOwnT = nc.dram_tensor("xOwnT", [DIN, NPC], BF, kind="ExternalInput")
    w1 = nc.dram_tensor("w1", [DIN, 2 * HID], BF, kind="ExternalInput")
    w2 = nc.dram_tensor("w2", [HID, 2 * DOUT], BF, kind="ExternalInput")
    att1r = nc.dram_tensor("att1r", [P, HID], BF, kind="ExternalInput")
    att2r = nc.dram_tensor("att2r", [P, DOUT], F32, kind="ExternalInput")
    identr = nc.dram_tensor("identr", [P, P], BF, kind="ExternalInput")
    idlo_d = nc.dram_tensor("idlo", [P, slots_lo // 16], mybir.dt.int16, kind="ExternalInput")
    idhi_d = nc.dram_tensor("idhi", [P, slots_hi // 16], mybir.dt.int16, kind="ExternalInput")
    mask_d = nc.dram_tensor("maskd", [P, tot_cols], F32, kind="ExternalInput")
    out_d = nc.dram_tensor("out", [NPC, DOUT], F32, kind="ExternalOutput")

    NKC = DIN // P

    with tile.TileContext(nc) as tc:
        with (
            tc.tile_pool(name="const", bufs=1) as cpool,
            tc.tile_pool(name="xk", bufs=2) as xkpool,
            tc.tile_pool(name="stg", bufs=3) as stgpool,
            tc.tile_pool(name="ps", bufs=2, space="PSUM") as pspool,
            tc.tile_pool(name="slab", bufs=1) as slabpool,
            tc.tile_pool(name="gs", bufs=2) as gspool,
            tc.tile_pool(name="ed", bufs=2) as edpool,
            tc.tile_pool(name="sm", bufs=2) as smpool,
            tc.tile_pool(name="dram", bufs=1, space="DRAM") as dpool,
        ):
            nc.gpsimd.load_library(library_config.mlp)
            xl_full_t = dpool.tile([NP, HID], BF, name="xl_full")
            xl2_loc_t = dpool.tile([NPC, DOUT], F32, name="xl2_loc")
            xl2_full_t = dpool.tile([NP, DOUT], F32, name="xl2_full", addr_space="Shared")

            w1s = cpool.tile([P, NKC * 2 * HID], BF, name="w1s")
            for k in range(NKC):
                nc.sync.dma_start(
                    w1s[:, k * 2 * HID : (k + 1) * 2 * HID],
                    w1.ap()[k * P : (k + 1) * P, :],
                )
            w2s = cpool.tile([P, 2 * 2 * DOUT], BF, name="w2s")
            for k in range(2):
                nc.sync.dma_start(
                    w2s[:, k * 2 * DOUT : (k + 1) * 2 * DOUT],
                    w2.ap()[k * P : (k + 1) * P, :],
                )
            att1s = cpool.tile([P, HID], BF, name="att1s")
            nc.sync.dma_start(att1s[:], att1r.ap())
            att2s = cpool.tile([P, DOUT], F32, name="att2s")
            nc.sync.dma_start(att2s[:], att2r.ap())
            ident = cpool.tile([P, P], BF, name="ident")
            nc.sync.dma_start(ident[:], identr.ap())
            shiftc = cpool.tile([P, 1], F32, name="shiftc")
            nc.vector.memset(shiftc[:], -SHIFT)
            masks = cpool.tile([P, tot_cols], F32, name="masks")
            nc.sync.dma_start(masks[:], mask_d.ap())

            xr_slab = slabpool.tile([P, NSTRIPE * HID], BF, name="xr_slab")
            h_slab = slabpool.tile([P, NSTRIPE * HID], BF, name="h_slab")
            xr2_slab = slabpool.tile([P, NSTRIPE * DOUT], F32, name="xr2_slab")

            # ---------- Phase 1a: replicated xl GEMM ----------
            GB = 7  # store-group blocks
            n_xch = NBLK // XCH_BLKS  # 28
            for ch in range(n_xch):
                b0 = ch * XCH_BLKS
                xk = xkpool.tile([P, NKC * XCH_BLKS * P], BF, name=f"xk{ch}", tag="xk")
                for k in range(NKC):
                    nc.sync.dma_start(
                        xk[:, k * XCH_BLKS * P : (k + 1) * XCH_BLKS * P],
                        xT.ap()[k * P : (k + 1) * P, b0 * P : (b0 + XCH_BLKS) * P],
                    )
                for gi in range(XCH_BLKS // GB):
                    stg = stgpool.tile([P, GB * HID], BF, name=f"st{ch}_{gi}", tag="stg")
                    for bj in range(GB):
                        bi = gi * GB + bj
                        ps = pspool.tile([P, HID], F32, name=f"ps{ch}_{gi}_{bj}", tag="gps")
                        for k in range(NKC):
                            nc.tensor.matmul(
                                ps[:],
                                xk[:, (k * XCH_BLKS + bi) * P : (k * XCH_BLKS + bi + 1) * P],
                                w1s[:, k * 2 * HID : k * 2 * HID + HID],
                                start=(k == 0),
                                stop=(k == NKC - 1),
                            )
                        nc.scalar.copy(stg[:, bj * HID : (bj + 1) * HID], ps[:])
                    gb0 = b0 + gi * GB
                    nc.sync.dma_start(
                        xl_full_t[gb0 * P : (gb0 + GB) * P, :].rearrange(
                            "(b p) w -> p b w", p=P
                        ),
                        stg[:].rearrange("p (b w) -> p b w", w=HID),
                    )

            # ---------- Phase 1b: local xr GEMM ----------
            for ch in range(NSTRIPE // 7):  # 7 chunks of 7 stripes
                b0 = ch * 7
                xo = xkpool.tile([P, NKC * 7 * P], BF, name=f"xo{ch}", tag="xk")
                for k in range(NKC):
                    nc.sync.dma_start(
                        xo[:, k * 7 * P : (k + 1) * 7 * P],
                        xOwnT.ap()[k * P : (k + 1) * P, b0 * P : (b0 + 7) * P],
                    )
                for bj in range(7):
                    j = b0 + bj
                    ps = pspool.tile([P, HID], F32, name=f"psr{ch}_{bj}", tag="gps")
                    for k in range(NKC):
                        nc.tensor.matmul(
                            ps[:],
                            xo[:, (k * 7 + bj) * P : (k * 7 + bj + 1) * P],
                            w1s[:, k * 2 * HID + HID : (k + 1) * 2 * HID],
                            start=(k == 0),
                            stop=(k == NKC - 1),
                        )
                    nc.scalar.copy(xr_slab[:, j * HID : (j + 1) * HID], ps[:])

            # ---------- Phase 2: L1 edge stage ----------
            def edge_layer(tab_lo, tab_hi, W, xrs, atts, ex_dt, res_dt, out_cb):
                """W: channels per edge (HID or DOUT); xrs: xr slab tile;
                out_cb(j, numer_ap, dr_ap): consume result per stripe."""
                off_lo = 0  # running slot offsets (elements of idx arrays)
                off_hi = 0
                mask_base = 0
                gdt = BF if W == HID else F32
                for ci, (s0, s1, cols) in enumerate(chunks):
                    lc = meta["chunk_lo_cols"][ci]
                    hc = meta["chunk_hi_cols"][ci]
                    gslab = gspool.tile(
                        [P, max_cols * W], gdt, name=f"gs{W}_{ci}", tag=f"gs{W}"
                    )
                    for (tab, nslots, ioff, idxd, q) in (
                        (tab_lo, lc * P, off_lo, idlo_d, 0),
                        (tab_hi, hc * P, off_hi, idhi_d, 1),
                    ):
                        if nslots == 0:
                            continue
                        it = smpool.tile(
                            [P, (max_cols * P) // 16],
                            mybir.dt.int16,
                            name=f"it{W}_{ci}_{q}",
                            tag=f"it{q}",
                        )
                        nc.sync.dma_start(
                            it[:, : nslots // 16],
                            idxd.ap()[:, ioff // 16 : (ioff + nslots) // 16],
                        )
                        col0 = 0 if q == 0 else lc
                        nc.gpsimd.dma_gather(
                            out_ap=gslab[:, col0 * W : (col0 * W + (nslots // P) * W)].rearrange(
                                "p (c w) -> p c w", w=W
                            ),
                            in_ap=tab,
                            idxs_ap=it[:, : nslots // 16],
                            num_idxs=nslots,
                            num_idxs_reg=nslots,
                            elem_size=W,
                            single_packet=False,
                            queue_num=q if W == HID else q + 2,
                        )
                    off_lo += lc * P
                    off_hi += hc * P
                    # per-stripe compute
                    for j in range(s0, s1):
                        dlo, dhi = int(Dlo[j]), int(Dhi[j])
                        D = dlo + dhi
                        if D == 0:
                            out_cb(j, None, None)
                            continue
                        # column ranges of stripe j in this chunk's slab
                        lo_o = meta["stripe_lo_off"][j]
                        hi_o = meta["stripe_hi_off"][j]
                        ranges = []
                        for (o, d) in ((lo_o, dlo), (hi_o, dhi)):
                            g0 = 0
                            while g0 < d:
                                gw = min(GW, d - g0)
                                ranges.append((o + g0, gw))
                                g0 += gw
                        numer_acc = smpool.tile([P, W], F32, name=f"na{W}_{j}", tag=f"na{W}")
                        den_acc = smpool.tile([P, H1 if W == HID else 1], F32, name=f"da{W}_{j}", tag=f"da{W}")
                        nh = H1 if W == HID else 1
                        wh = DH if W == HID else DOUT
                        first = True
                        for (co, gw) in ranges:
                            xlg = gslab[:, co * W : (co + gw) * W]
                            g_t = edpool.tile([P, GW * W], gdt, name=f"g{W}_{j}_{co}", tag=f"g{W}")
                            nc.vector.tensor_tensor(
                                out=g_t[:, : gw * W].rearrange("p (d w) -> p d w", w=W),
                                in0=xlg.rearrange("p (d w) -> p d w", w=W),
                                in1=_apx(xrs[:, j * W : (j + 1) * W], [(0, gw), (1, W)]),
                                op=mybir.AluOpType.add,
                            )
                            l0 = edpool.tile([P, GW * W], gdt, name=f"l0{W}_{j}_{co}", tag=f"l0{W}")
                            nc.vector.tensor_scalar(
                                out=l0[:, : gw * W], in0=g_t[:, : gw * W],
                                scalar1=NEG, scalar2=None, op0=mybir.AluOpType.mult,
                            )
                            lg = edpool.tile([P, GW * W], gdt, name=f"lg{W}_{j}_{co}", tag=f"lg{W}")
                            nc.vector.tensor_tensor(
                                out=lg[:, : gw * W], in0=g_t[:, : gw * W],
                                in1=l0[:, : gw * W], op=mybir.AluOpType.max,
                            )
                            prod = edpool.tile([P, GW * W], gdt, name=f"pr{W}_{j}_{co}", tag=f"pr{W}")
                            nc.vector.tensor_tensor(
                                out=prod[:, : gw * W].rearrange("p (d w) -> p d w", w=W),
                                in0=lg[:, : gw * W].rearrange("p (d w) -> p d w", w=W),
                                in1=_apx(atts[:], [(0, gw), (1, W)]),
                                op=mybir.AluOpType.mult,
                            )
                            e4 = smpool.tile([P, GW * H1], F32, name=f"e4{W}_{j}_{co}", tag=f"e4{W}")
                            nc.vector.tensor_reduce(
                                out=e4[:, : gw * nh].rearrange("p (d h) -> p d h", h=nh),
                                in_=prod[:, : gw * W].rearrange(
                                    "p (d h w) -> p d h w", h=nh, w=wh
                                ),
                                axis=mybir.AxisListType.X,
                                op=mybir.AluOpType.add,
                            )
                            exf = smpool.tile([P, GW * H1], F32, name=f"ex{W}_{j}_{co}", tag=f"exf{W}")
                            nc.scalar.activation(
                                exf[:, : gw * nh], e4[:, : gw * nh],
                                mybir.ActivationFunctionType.Exp, bias=shiftc[:, 0:1],
                            )
                            exm = smpool.tile([P, GW * H1], F32, name=f"em{W}_{j}_{co}", tag=f"exm{W}")
                            nc.vector.tensor_tensor(
                                out=exm[:, : gw * nh].rearrange("p (d h) -> p d h", h=nh),
                                in0=exf[:, : gw * nh].rearrange("p (d h) -> p d h", h=nh),
                                in1=_apx(masks[:, mask_base + co : mask_base + co + gw], [(1, gw), (0, nh)]),
                                op=mybir.AluOpType.mult,
                            )
                            dpart_t = smpool.tile([P, H1 if W == HID else 1], F32, name=f"dp{W}_{j}_{co}", tag=f"dp{W}")
                            nc.vector.tensor_reduce(
                                out=dpart_t[:],
                                in_=_apx(exm[:], [(1, nh), (nh, gw)]),
                                axis=mybir.AxisListType.X,
                                op=mybir.AluOpType.add,
                            )
                            exb = smpool.tile([P, GW * H1], gdt, name=f"eb{W}_{j}_{co}", tag=f"exb{W}")
                            nc.vector.tensor_copy(exb[:, : gw * nh], exm[:, : gw * nh])
                            w_t = edpool.tile([P, GW * W], gdt, name=f"w{W}_{j}_{co}", tag=f"w{W}")
                            nc.vector.tensor_tensor(
                                out=w_t[:, : gw * W].rearrange(
                                    "p (d h w) -> p d h w", h=nh, w=wh
                                ),
                                in0=xlg.rearrange("p (d h w) -> p d h w", h=nh, w=wh),
                                in1=_apx(exb[:], [(nh, gw), (1, nh), (0, wh)]),
                                op=mybir.AluOpType.mult,
                            )
                            npart = smpool.tile([P, W], F32, name=f"np{W}_{j}_{co}", tag=f"np{W}")
                            nc.vector.tensor_reduce(
                                out=npart[:],
                                in_=_apx(w_t[:], [(1, W), (W, gw)]),
                                axis=mybir.AxisListType.X,
                                op=mybir.AluOpType.add,
                            )
                            if first:
                                nc.vector.tensor_copy(numer_acc[:], npart[:])
                                nc.vector.tensor_copy(den_acc[:], dpart_t[:])
                                first = False
                            else:
                                nc.vector.tensor_tensor(
                                    out=numer_acc[:], in0=numer_acc[:], in1=npart[:],
                                    op=mybir.AluOpType.add,
                                )
                                nc.vector.tensor_tensor(
                                    out=den_acc[:], in0=den_acc[:], in1=dpart_t[:],
                                    op=mybir.AluOpType.add,
                                )
                        den2 = smpool.tile([P, H1 if W == HID else 1], F32, name=f"d2{W}_{j}", tag=f"d2{W}")
                        nc.vector.tensor_scalar(
                            out=den2[:], in0=den_acc[:], scalar1=1e-16, scalar2=None,
                            op0=mybir.AluOpType.add,
                        )
                        dr = smpool.tile([P, H1 if W == HID else 1], F32, name=f"dr{W}_{j}", tag=f"dr{W}")
                        nc.vector.reciprocal(dr[:], den2[:])
                        out_cb(j, numer_acc, dr)
                    mask_base += cols

            # L1 consumer: h = elu(numer*dr) -> h_slab
            def l1_out(j, numer, dr):
                dst = h_slab[:, j * HID : (j + 1) * HID]
                if numer is None:
                    nc.vector.memset(dst, 0.0)
                    return
                hb = smpool.tile([P, HID], F32, name=f"hb{j}", tag="hb")
                nc.vector.tensor_tensor(
                    out=hb[:].rearrange("p (h w) -> p h w", h=H1),
                    in0=numer[:].rearrange("p (h w) -> p h w", h=H1),
                    in1=_apx(dr[:], [(1, H1), (0, DH)]),
                    op=mybir.AluOpType.mult,
                )
                mn = smpool.tile([P, HID], F32, name=f"mn{j}", tag="mn")
                nc.vector.tensor_scalar(
                    out=mn[:], in0=hb[:], scalar1=0.0, scalar2=None,
                    op0=mybir.AluOpType.min,
                )
                en = smpool.tile([P, HID], F32, name=f"en{j}", tag="en")
                nc.scalar.activation(en[:], mn[:], mybir.ActivationFunctionType.Exp)
                rp = smpool.tile([P, HID], F32, name=f"rp{j}", tag="rp")
                nc.vector.tensor_scalar(
                    out=rp[:], in0=hb[:], scalar1=0.0, scalar2=None,
                    op0=mybir.AluOpType.max,
                )
                s1 = smpool.tile([P, HID], F32, name=f"s1{j}", tag="s1")
                nc.vector.tensor_tensor(
                    out=s1[:], in0=rp[:], in1=en[:], op=mybir.AluOpType.add
                )
                nc.vector.tensor_scalar(
                    out=dst, in0=s1[:], scalar1=-1.0, scalar2=None,
                    op0=mybir.AluOpType.add,
                )

            edge_layer(
                xl_full_t[0:SPLIT, :],
                xl_full_t[SPLIT:NP, :],
                HID,
                xr_slab,
                att1s,
                F32,
                F32,
                l1_out,
            )

            # ---------- Phase 2b: L2 GEMM per stripe ----------
            for j in range(NSTRIPE):
                hT = stgpool.tile([P, 2 * P], BF, name=f"hT{j}", tag="hT")
                for half in range(2):
                    tp = pspool.tile([P, P], BF, name=f"tp{j}_{half}", tag="tp")
                    nc.tensor.transpose(
                        out=tp[:],
                        in_=h_slab[:, j * HID + half * P : j * HID + (half + 1) * P],
                        identity=ident[:],
                    )
                    nc.scalar.copy(hT[:, half * P : (half + 1) * P], tp[:])
                ps2 = pspool.tile([P, 2 * DOUT], F32, name=f"ps2{j}", tag="ps2")
                for k in range(2):
                    nc.tensor.matmul(
                        ps2[:],
                        hT[:, k * P : (k + 1) * P],
                        w2s[:, k * 2 * DOUT : (k + 1) * 2 * DOUT],
                        start=(k == 0),
                        stop=(k == 1),
                    )
                xl2b = smpool.tile([P, DOUT], F32, name=f"x2b{j}", tag="x2b")
                nc.scalar.copy(xl2b[:], ps2[:, :DOUT])
                nc.sync.dma_start(xl2_loc_t[j * P : (j + 1) * P, :], xl2b[:])
                nc.scalar.copy(xr2_slab[:, j * DOUT : (j + 1) * DOUT], ps2[:, DOUT:])

            # ---------- Phase 3: AllGather xl2 ----------
            nc.gpsimd.collective_compute(
                "AllGather",
                mybir.AluOpType.bypass,
                ins=[xl2_loc_t[:].opt()],
                outs=[xl2_full_t[:].opt()],
                replica_groups=[list(range(NC))],
            )

            # ---------- Phase 4: L2 edge + log_softmax ----------
            def l2_out(j, numer, dr):
                h2 = smpool.tile([P, DOUT], F32, name=f"h2{j}", tag="h2")
                if numer is None:
                    nc.vector.memset(h2[:], 0.0)
                else:
                    nc.vector.tensor_scalar(
                        out=h2[:], in0=numer[:], scalar1=dr[:, 0:1], scalar2=None,
                        op0=mybir.AluOpType.mult,
                    )
                mx = smpool.tile([P, 1], F32, name=f"mx{j}", tag="mx")
                nc.vector.tensor_reduce(
                    out=mx[:], in_=h2[:], axis=mybir.AxisListType.X,
                    op=mybir.AluOpType.max,
                )
                sx = smpool.tile([P, DOUT], F32, name=f"sx{j}", tag="sx")
                nc.vector.tensor_scalar(
                    out=sx[:], in0=h2[:], scalar1=mx[:, 0:1], scalar2=None,
                    op0=mybir.AluOpType.subtract,
                )
                es = smpool.tile([P, DOUT], F32, name=f"es{j}", tag="es")
                sm = smpool.tile([P, 1], F32, name=f"sm{j}", tag="smt")
                nc.scalar.activation(
                    es[:], sx[:], mybir.ActivationFunctionType.Exp, accum_out=sm[:]
                )
                ln = smpool.tile([P, 1], F32, name=f"ln{j}", tag="ln")
                nc.scalar.activation(ln[:], sm[:], mybir.ActivationFunctionType.Ln)
                lso = smpool.tile([P, DOUT], F32, name=f"lso{j}", tag="lso")
                nc.vector.tensor_scalar(
                    out=lso[:], in0=sx[:], scalar1=ln[:, 0:1], scalar2=None,
                    op0=mybir.AluOpType.subtract,
                )
                nc.sync.dma_start(out_d.ap()[j * P : (j + 1) * P, :], lso[:])

            edge_layer(
                xl2_full_t[0:SPLIT, :],
                xl2_full_t[SPLIT:NP, :],
                DOUT,
                xr2_slab,
                att2s,
                F32,
                F32,
                l2_out,
            )

    nc.compile()
    return nc


_CACHE = {}
LAST_EXEC_NS = None
LAST_RES = None


def kernel(**inputs):
    x = np.asarray(inputs["x"], dtype=np.float32)
    edge_index = np.asarray(inputs["edge_index"])
    W1l = np.asarray(inputs["W1l"], dtype=np.float32)
    W1r = np.asarray(inputs["W1r"], dtype=np.float32)
    att1 = np.asarray(inputs["att1"], dtype=np.float32)
    W2l = np.asarray(inputs["W2l"], dtype=np.float32)
    W2r = np.asarray(inputs["W2r"], dtype=np.float32)
    att2 = np.asarray(inputs["att2"], dtype=np.float32)

    per_core, meta = _host_prep(edge_index)
    nc = _gen_nc(meta)
    _spill_waits(nc)

    old_of_nid = meta["old_of_nid"]
    xp = np.zeros((NP, DIN), dtype=np.float32)
    xp[:N] = x
    x_re = xp[old_of_nid]  # relabeled rows
    xT_np = np.ascontiguousarray(x_re.T).astype(NPBF)
    w1_np = np.concatenate([W1l, W1r], axis=1).astype(NPBF)
    w2_np = np.concatenate([W2l, W2r], axis=1).astype(NPBF)
    att1r_np = np.tile(att1.reshape(1, HID), (P, 1)).astype(NPBF)
    att2r_np = np.tile(att2.reshape(1, DOUT), (P, 1)).astype(np.float32)
    ident_np = np.eye(P, dtype=np.float32).astype(NPBF)

    in_maps = []
    for c in range(NC):
        grid, mask = per_core[c]
        ilo, ihi = _wrap_idx_streams(grid, meta)
        in_maps.append(
            {
                "xT": xT_np,
                "xOwnT": np.ascontiguousarray(xT_np[:, c * NPC : (c + 1) * NPC]),
                "w1": w1_np,
                "w2": w2_np,
                "att1r": att1r_np,
                "att2r": att2r_np,
                "identr": ident_np,
                "idlo": ilo,
                "idhi": ihi,
                "maskd": mask,
            }
        )

    import os
    trace = os.environ.get("KERNEL_TRACE") == "1"
    if trace:
        import axon_profile_shim  # noqa: F401
    res = run_bass_kernel_spmd(nc, in_maps, core_ids=list(range(NC)), trace=trace)
    global LAST_EXEC_NS, LAST_RES
    LAST_EXEC_NS = res.exec_time_ns
    LAST_RES = res
    out = np.zeros((N, DOUT), dtype=np.float32)
    for c in range(NC):
        oc = res.results[c]["out"]
        olds = old_of_nid[c * NPC : (c + 1) * NPC]
        sel = olds < N
        out[olds[sel]] = oc[sel]
    return out
